# revision 6
# baseline (speedup 1.0000x reference)
"""AdaptiveNormalization Trainium2 kernel (8 NeuronCores, batch-parallel).

Reference computation (per batch b):
    a      = ema(x, m)                      # causal EMA over T, per (b,c)
    shift  = sum_c w_shift[c] * a[c,t]      # (b,t)
    x1     = x - shift
    bb     = ema(x1^2, m)
    scale  = sum_c exp(w_scale_log)[c] * bb[c,t]
    out    = (x1 / sqrt(scale+eps)) * w_proj[c] + b_proj[c]

Key rewrite: the EMA is a linear time-invariant filter applied identically to
every channel, so the channel reduction commutes with it:
    shift = ema(s),  s_t = sum_c w_shift[c] x[c,t]
    scale = ema(q),  q_t = u_t - 2*shift_t*v_t + shift_t^2*E
        u = sum_c e_c x^2,  v = sum_c e_c x,  E = sum_c e_c
This collapses 2*B*C length-T scans into 2*B scalar scans, which are computed
with hardware tensor_tensor_scan on a [128 blocks x 128] layout plus a single
matmul for the inter-block carry.

Final output:  out = x (.) A + B  with rank-1 A = w_proj (x) inv and
rank-2 B = b_proj (x) 1 - w_proj (x) (shift*inv), both produced by tiny
PE matmuls tile by tile.
"""

import sys
import os

for _p in ("/opt/trn_rl_repo",):
    if _p not in sys.path:
        sys.path.insert(0, _p)

import numpy as np
from contextlib import ExitStack

import concourse.bass as bass
import concourse.bacc as bacc
import concourse.tile as tile
from concourse import mybir
from concourse import bass_utils

MOMENTUM = 0.01
EPS = 1e-6
B, C, T_FULL = 8, 256, 16384
N_CORES = 8
BS = 128          # scan block size (elements per block, on the free dim)

F32 = mybir.dt.float32
F32R = mybir.dt.float32r
AOP = mybir.AluOpType
ACTF = mybir.ActivationFunctionType


def _host_constants(w_shift, w_scale_log, w_proj, b_proj, T):
    """Host-side folded weights + scan constants (all float32)."""
    m = MOMENTUM
    r = 1.0 - m
    NB = T // BS
    ws = w_shift.astype(np.float64)
    e = np.exp(w_scale_log.astype(np.float64))
    wp = w_proj.astype(np.float64)
    bp = b_proj.astype(np.float64)

    w_sv = np.zeros((128, 4), np.float64)
    w_sv[:, 0] = m * ws[:128]
    w_sv[:, 1] = -2.0 * m * e[:128]
    w_sv[:, 2] = m * ws[128:]
    w_sv[:, 3] = -2.0 * m * e[128:]

    w_u = np.zeros((128, 4), np.float64)
    w_u[:, 0] = m * e[:128]
    w_u[:, 2] = m * e[128:]

    w_a = wp.reshape(1, 256)
    w_b = np.stack([bp, -wp], 0)  # (2, 256)

    # carry matrix: c'[j] = sum_{k<=j-1} (r^BS)^(j-1-k) * ylast[k]
    # stored as lhsT: l2p[k, j] = (r^BS)^(j-1-k) for k <= j-1 else 0
    kk = np.arange(NB)[:, None]
    jj = np.arange(NB)[None, :]
    expo = (jj - 1 - kk).astype(np.float64)
    l2p = np.where(expo >= 0, (r ** BS) ** np.maximum(expo, 0.0), 0.0)

    rvec = (r ** (np.arange(BS, dtype=np.float64) + 1.0))[None, :].repeat(NB, 0)

    ecol = np.full((NB, 1), m * e.sum(), np.float64)
    ones_r = np.ones((1, T), np.float32)

    f = lambda a: np.ascontiguousarray(a, dtype=np.float32)
    return dict(
        w_sv=f(w_sv), w_u=f(w_u), w_a=f(w_a), w_b=f(w_b),
        l2p_t=f(l2p), rvec_b=f(rvec), e_col=f(ecol), ones_r=ones_r,
    )


def build_model(T=T_FULL):
    """Build the per-core Bass graph (SPMD; identical on all cores)."""
    m = MOMENTUM
    r = 1.0 - m
    NB = T // BS
    XCH = min(4096, T)        # x load chunk (free dim)
    NG = T // 1024            # phase-1 groups (1024 T-cols each)
    QS = min(2048, T)         # phase-3 row-chunk size
    NQ = T // QS

    nc = bacc.Bacc("TRN2", target_bir_lowering=False, debug=False)

    # Tensors feeding fp32r matmuls are declared float32r end-to-end
    # (walrus requires fp32r matmul inputs to be produced as fp32r);
    # non-matmul consumers read them via .bitcast(F32).
    x_d = nc.dram_tensor("x", [C, T], F32R, kind="ExternalInput")
    wsv_d = nc.dram_tensor("w_sv", [128, 4], F32R, kind="ExternalInput")
    wu_d = nc.dram_tensor("w_u", [128, 4], F32R, kind="ExternalInput")
    wa_d = nc.dram_tensor("w_a", [1, 256], F32R, kind="ExternalInput")
    wb_d = nc.dram_tensor("w_b", [2, 256], F32R, kind="ExternalInput")
    l2p_d = nc.dram_tensor("l2p_t", [NB, NB], F32, kind="ExternalInput")
    rvec_d = nc.dram_tensor("rvec_b", [NB, BS], F32, kind="ExternalInput")
    ecol_d = nc.dram_tensor("e_col", [NB, 1], F32, kind="ExternalInput")
    ones_d = nc.dram_tensor("ones_r", [1, T], F32R, kind="ExternalInput")
    out_d = nc.dram_tensor("out", [C, T], F32, kind="ExternalOutput")

    with tile.TileContext(nc) as tc, ExitStack() as ctx:
        consts = ctx.enter_context(tc.tile_pool(name="consts", bufs=1))
        xpool = ctx.enter_context(tc.tile_pool(name="x", bufs=1))
        scanp = ctx.enter_context(tc.tile_pool(name="scan", bufs=1))
        dpool = ctx.enter_context(tc.tile_pool(name="dram", bufs=1, space="DRAM"))

        # ---- constants to SBUF ----
        wsv_sb = consts.tile([128, 4], F32R)
        wu_sb = consts.tile([128, 4], F32R)
        wa_sb = consts.tile([1, 256], F32R)
        wb_sb = consts.tile([2, 256], F32R)
        l2p_sb = consts.tile([NB, NB], F32)
        rvec_sb = consts.tile([NB, BS], F32)
        ecol_sb = consts.tile([NB, 1], F32)
        rfill = consts.tile([NB, BS], F32)
        eps_sb = consts.tile([NB, 1], F32)
        nc.vector.memset(eps_sb[:], EPS)
        for sb, d in ((wsv_sb, wsv_d), (wu_sb, wu_d), (wa_sb, wa_d),
                      (wb_sb, wb_d), (l2p_sb, l2p_d), (rvec_sb, rvec_d),
                      (ecol_sb, ecol_d)):
            nc.sync.dma_start(sb[:], d[:])
        nc.vector.memset(rfill[:], r)

        # full-batch x resident in SBUF: [128 part, half, T]
        x_sb = xpool.tile([128, 2, T], F32R)
        for ch in range(T // XCH):
            sl = slice(ch * XCH, (ch + 1) * XCH)
            for h in (0, 1):
                nc.sync.dma_start(x_sb[:, h, sl], x_d[h * 128:(h + 1) * 128, sl])

        # DRAM scratch rows: 0=s', 1=v'', 2=u', 3=inv, 4=shift*inv
        rows_d = dpool.tile([5, T], F32R)

        # ---- phase 1: per-t channel reductions s', v'', u' ----
        with tc.tile_pool(name="sq", bufs=2) as sqpool, \
             tc.tile_pool(name="stage", bufs=3) as stpool, \
             tc.tile_pool(name="ps1", bufs=2, space="PSUM") as ps1:
            for g in range(NG):
                g0 = g * 1024
                svu_ps = ps1.tile([2, 2048], F32, tag="svu")
                for h in (0, 1):
                    sq = sqpool.tile([128, 1024], F32R, tag=f"sq{h}")
                    xg = x_sb[:, h, g0:g0 + 1024].bitcast(F32)
                    nc.gpsimd.tensor_tensor(sq[:], xg, xg, AOP.mult)
                    for k in (0, 1):
                        xsl = x_sb[:, h, g0 + k * 512:g0 + (k + 1) * 512]
                        nc.tensor.matmul(
                            svu_ps[:, k * 512:(k + 1) * 512],
                            lhsT=wsv_sb[:, 2 * h:2 * h + 2],
                            rhs=xsl,
                            start=(h == 0), stop=(h == 1))
                        nc.tensor.matmul(
                            svu_ps[:, 1024 + k * 512:1024 + (k + 1) * 512],
                            lhsT=wu_sb[:, 2 * h:2 * h + 2],
                            rhs=sq[:, k * 512:(k + 1) * 512],
                            start=(h == 0), stop=(h == 1))
                stage = stpool.tile([2, 2048], F32, tag="stage")
                nc.scalar.copy(stage[:], svu_ps[:])
                gsl = slice(g0, g0 + 1024)
                nc.sync.dma_start(rows_d[0:1, gsl].bitcast(F32), stage[0:1, 0:1024])
                nc.sync.dma_start(rows_d[1:2, gsl].bitcast(F32), stage[1:2, 0:1024])
                nc.sync.dma_start(rows_d[2:3, gsl].bitcast(F32), stage[0:1, 1024:2048])

        # ---- phase 2: scalar EMA scans in [NB, BS] block layout ----
        with tc.tile_pool(name="ps2", bufs=2, space="PSUM") as ps2:
            S_s = scanp.tile([NB, BS], F32)
            S_v = scanp.tile([NB, BS], F32)
            S_u = scanp.tile([NB, BS], F32)
            nc.sync.dma_start(
                S_s[:],
                rows_d[0:1, :].bitcast(F32).rearrange("p (b i) -> (p b) i", b=NB))
            nc.sync.dma_start(
                S_v[:],
                rows_d[1:2, :].bitcast(F32).rearrange("p (b i) -> (p b) i", b=NB))
            nc.sync.dma_start(
                S_u[:],
                rows_d[2:3, :].bitcast(F32).rearrange("p (b i) -> (p b) i", b=NB))

            def ema_scan(src, out_name):
                """Blockwise scan + inter-block carry: returns fixed-up tile."""
                loc = scanp.tile([NB, BS], F32, tag=f"{out_name}_loc")
                nc.vector.tensor_tensor_scan(
                    loc[:], rfill[:], src[:], 0.0, AOP.mult, AOP.add)
                c_ps = ps2.tile([NB, 1], F32, tag=f"{out_name}_cps")
                nc.tensor.matmul(c_ps[:], lhsT=l2p_sb[:],
                                 rhs=loc[:, BS - 1:BS], start=True, stop=True)
                c_sb = scanp.tile([NB, 1], F32, tag=f"{out_name}_c")
                nc.vector.tensor_copy(c_sb[:], c_ps[:])
                fixed = scanp.tile([NB, BS], F32, tag=f"{out_name}_fix")
                nc.vector.scalar_tensor_tensor(
                    fixed[:], rvec_sb[:], c_sb[:], loc[:], AOP.mult, AOP.add)
                return fixed

            shift_S = ema_scan(S_s, "shift")

            # m*q = u' + shift*v'' + shift^2 * (m*E)
            t1 = scanp.tile([NB, BS], F32)
            nc.vector.tensor_tensor(t1[:], shift_S[:], S_v[:], AOP.mult)
            t2 = scanp.tile([NB, BS], F32)
            nc.vector.tensor_tensor(t2[:], shift_S[:], shift_S[:], AOP.mult)
            q1 = scanp.tile([NB, BS], F32)
            nc.vector.scalar_tensor_tensor(
                q1[:], t2[:], ecol_sb[:], S_u[:], AOP.mult, AOP.add)
            qm = scanp.tile([NB, BS], F32)
            nc.vector.tensor_tensor(qm[:], q1[:], t1[:], AOP.add)

            scale_S = ema_scan(qm, "scale")

            sq_s = scanp.tile([NB, BS], F32)
            nc.scalar.activation(sq_s[:], scale_S[:], ACTF.Sqrt, bias=eps_sb[:])
            inv_S = scanp.tile([NB, BS], F32R)
            nc.vector.reciprocal(inv_S[:].bitcast(F32), sq_s[:])
            si_S = scanp.tile([NB, BS], F32R)
            nc.vector.tensor_tensor(
                si_S[:].bitcast(F32), shift_S[:], inv_S[:].bitcast(F32), AOP.mult)

            nc.sync.dma_start(
                rows_d[3:4, :].rearrange("p (b i) -> (p b) i", b=NB), inv_S[:])
            nc.sync.dma_start(
                rows_d[4:5, :].rearrange("p (b i) -> (p b) i", b=NB), si_S[:])

        # ---- phase 3: out = x .* A + B ----
        with tc.tile_pool(name="rows", bufs=1) as rowp, \
             tc.tile_pool(name="z", bufs=3) as zpool, \
             tc.tile_pool(name="o", bufs=3) as opool, \
             tc.tile_pool(name="ps3", bufs=2, space="PSUM") as ps3:
            for o in range(NQ):
                osl = slice(o * QS, (o + 1) * QS)
                rab = rowp.tile([2, QS], F32R, tag="rab")
                ra = rowp.tile([1, QS], F32R, tag="ra")
                nc.sync.dma_start(rab[0:1, :], ones_d[0:1, osl])
                nc.sync.dma_start(rab[1:2, :], rows_d[4:5, osl])
                nc.sync.dma_start(ra[:], rows_d[3:4, osl])
                for w in range(QS // 1024):
                    for h in (0, 1):
                        a_ps = ps3.tile([128, 1024], F32, tag="aps")
                        b_ps = ps3.tile([128, 1024], F32, tag="bps")
                        for k in (0, 1):
                            lsl = slice(w * 1024 + k * 512,
                                        w * 1024 + (k + 1) * 512)
                            nc.tensor.matmul(
                                a_ps[:, k * 512:(k + 1) * 512],
                                lhsT=wa_sb[0:1, h * 128:(h + 1) * 128],
                                rhs=ra[0:1, lsl],
                                start=True, stop=True)
                            nc.tensor.matmul(
                                b_ps[:, k * 512:(k + 1) * 512],
                                lhsT=wb_sb[0:2, h * 128:(h + 1) * 128],
                                rhs=rab[0:2, lsl],
                                start=True, stop=True)
                        gsl = slice(o * QS + w * 1024, o * QS + (w + 1) * 1024)
                        z = zpool.tile([128, 1024], F32, tag="z")
                        nc.vector.tensor_tensor(
                            z[:], x_sb[:, h, gsl].bitcast(F32), a_ps[:], AOP.mult)
                        ot = opool.tile([128, 1024], F32, tag="ot")
                        nc.vector.tensor_tensor(ot[:], z[:], b_ps[:], AOP.add)
                        nc.sync.dma_start(
                            out_d[h * 128:(h + 1) * 128, gsl], ot[:])

    nc.compile()
    return nc


_MODEL_CACHE = {}


def _get_model(T=T_FULL):
    if T not in _MODEL_CACHE:
        _MODEL_CACHE[T] = build_model(T)
    return _MODEL_CACHE[T]


def make_in_maps(x, w_shift, w_scale_log, w_proj, b_proj, T):
    """Per-core input dicts (core i gets batch i)."""
    consts = _host_constants(w_shift, w_scale_log, w_proj, b_proj, T)
    nb = x.shape[0]
    in_maps = []
    for i in range(nb):
        im = {"x": np.ascontiguousarray(x[i], dtype=np.float32)}
        im.update(consts)
        in_maps.append(im)
    return in_maps


def kernel(x, w_shift, w_scale_log, w_proj, b_proj):
    T = x.shape[-1]
    nc = _get_model(T)
    in_maps = make_in_maps(x, w_shift, w_scale_log, w_proj, b_proj, T)
    res = bass_utils.run_bass_kernel_spmd(
        nc, in_maps, core_ids=list(range(len(in_maps))))
    out = np.stack([res.results[i]["out"] for i in range(len(in_maps))], 0)
    return out.astype(np.float32)
